# revision 10
# baseline (speedup 1.0000x reference)
"""Additive (Bahdanau) attention on 8 Trainium2 NeuronCores.

Problem:  out[b,q,v] = softmax_k( sum_h wv[h] * tanh(qp[b,q,h] + kp[b,k,h]) ) @ V
          qp = queries @ Wq.T, kp = keys @ Wk.T, key-axis mask from valid_lens.
Shapes:   B=4, Q=256, K=1024, D=512, H=256.

Sharding: 8 shards = (batch b, query-half qh); each core handles 128 queries of
one batch. All projections/tanh/softmax/AV run on-device; the host only slices,
transposes, casts and zero-pads inputs (layout prep).

Device algorithm (per core):
  - qp (h-part, q-free) and kp (h-part, k-free) via fp32 PE matmuls (inputs
    fed d-major so the contraction dim sits on partitions).
  - For each query q and each h-tile t: ScalarE computes
    feats = tanh(kp_t + qp_t[:, q])  in ONE activation instruction
    (the per-partition `bias` operand does the broadcast add for free),
    writing fp16 (ScalarE rate is dtype-independent; fp16 keeps the PE
    score matmuls at full 1 cycle/row instead of fp32's 4).
  - PE reduces over h with a zero-padded fp16 wv weight tile: lhsT is
    (128h, 32) holding wv_t in column q%32, so the scores land in PSUM row q
    (output base partition q//32*32, 32-aligned as the PE requires). Matmuls
    accumulate into a (128q, 1024k) fp32 PSUM scores tile; zero weight
    columns contribute nothing to the other 31 rows of the strip.
  - softmax without max-subtraction: e = exp(scores - 6) in fp16.
    |score| <= sum|wv| ~ 13, so e <= e^7 ~ 1100 fits fp16; the constant
    shift cancels exactly in the normalization. Masking is folded into the
    AV step: V rows >= valid_len are zeroed on the host and the denominator
    uses a masked ones-vector, so no mask instructions run on-device.
  - e is PE-transposed (8x 128x128, fp16) and PE computes out = eT.T @ Vm
    and s = eT.T @ ones_masked (fp32 PSUM accumulation); DVE reciprocal +
    per-partition scale finish in fp32.
"""

import functools
import numpy as np

B, Q, K, D, H = 4, 256, 1024, 512, 256
NCORES = 8
QS = Q // 2  # queries per core (128)
NDT = 4      # d-tiles of 128 in D=512
NKT = K // 128
ESHIFT = 6.0  # exp(score - ESHIFT); cancels in normalization


def _emit(nc, tc, out, qT, kT, wqT, wkT, wv, v, ones, ident, cfg, reps=1):
    import concourse.mybir as mybir

    F32 = mybir.dt.float32
    F16 = mybir.dt.float16
    AF = mybir.ActivationFunctionType

    with (
        tc.tile_pool(name="consts", bufs=1) as consts,
        tc.tile_pool(name="work", bufs=2) as work,
        tc.tile_pool(name="feats", bufs=4) as featsp,
        tc.tile_pool(name="pproj", bufs=2, space="PSUM") as pproj,
        tc.tile_pool(name="pscore", bufs=1, space="PSUM") as pscore,
        tc.tile_pool(name="ptrans", bufs=2, space="PSUM") as ptrans,
        tc.tile_pool(name="pout", bufs=1, space="PSUM") as pout,
    ):
        # ---- load everything into SBUF (once) ----
        qts = consts.tile([128, NDT * QS], F32)
        for d in range(NDT):
            nc.sync.dma_start(qts[:, d * QS:(d + 1) * QS], qT[0, d])
        kts = consts.tile([128, NDT * K], F32)
        for d in range(NDT):
            nc.sync.dma_start(kts[:, d * K:(d + 1) * K], kT[0, d])
        wqts = consts.tile([128, NDT * H], F32)
        for d in range(NDT):
            nc.sync.dma_start(wqts[:, d * H:(d + 1) * H], wqT[0, d])
        wkts = consts.tile([128, NDT * H], F32)
        for d in range(NDT):
            nc.sync.dma_start(wkts[:, d * H:(d + 1) * H], wkT[0, d])
        wvt = consts.tile([128, 2048], F16)
        nc.sync.dma_start(wvt[:], wv[0])
        vts = consts.tile([128, NKT * D], F16)
        for kt in range(NKT):
            nc.sync.dma_start(vts[:, kt * D:(kt + 1) * D], v[0, kt])
        onest = consts.tile([128, NKT], F16)
        nc.sync.dma_start(onest[:], ones[0])
        idt = consts.tile([128, 128], F16)
        nc.sync.dma_start(idt[:], ident[0])
        shiftb = consts.tile([128, 1], F32)
        nc.vector.memset(shiftb[:], -ESHIFT)

        def one_rep():
            # ---- projections (fp32): qp (128h x 2t*128q), kp (128h x 2t*1024k)
            qp = work.tile([128, 2 * QS], F32, tag="qp")
            for t in range(2):
                ps = pproj.tile([128, 512], F32, tag="pp")
                for d in range(NDT):
                    nc.tensor.matmul(
                        ps[:, :QS],
                        wqts[:, d * H + t * 128: d * H + (t + 1) * 128],
                        qts[:, d * QS:(d + 1) * QS],
                        start=(d == 0), stop=(d == NDT - 1),
                    )
                nc.vector.tensor_copy(qp[:, t * QS:(t + 1) * QS], ps[:, :QS])
            kp = work.tile([128, 2 * K], F32, tag="kp")
            for t in range(2):
                for kh in range(2):
                    ps = pproj.tile([128, 512], F32, tag="pp")
                    for d in range(NDT):
                        nc.tensor.matmul(
                            ps[:],
                            wkts[:, d * H + t * 128: d * H + (t + 1) * 128],
                            kts[:, d * K + kh * 512: d * K + (kh + 1) * 512],
                            start=(d == 0), stop=(d == NDT - 1),
                        )
                    nc.vector.tensor_copy(
                        kp[:, t * K + kh * 512: t * K + (kh + 1) * 512], ps[:])

            # ---- query loop: tanh (fp16 out) + score accumulation (fp32 PSUM)
            sc = pscore.tile([128, 1024], F32, tag="sc")  # 2 PSUM banks
            for q in range(QS):
                j, c = q // 32, q % 32
                for t in range(2):
                    ft = featsp.tile([128, K], F16, tag="ft")
                    nc.scalar.activation(
                        ft[:], kp[:, t * K:(t + 1) * K], AF.Tanh,
                        bias=qp[:, t * QS + q: t * QS + q + 1], scale=1.0,
                    )
                    w = wvt[:, (t * 32 + c) * 32: (t * 32 + c) * 32 + 32]
                    for kh in range(2):
                        nc.tensor.matmul(
                            sc[32 * j: 32 * j + 32, kh * 512:(kh + 1) * 512],
                            w,
                            ft[:, kh * 512:(kh + 1) * 512],
                            start=(q % 32 == 0 and t == 0),
                            stop=(q % 32 == 31 and t == 1),
                            skip_group_check=True,
                            tile_position=(0, 32 * j),
                        )

            # ---- softmax (shifted, no max-subtraction) + AV ----
            e = work.tile([128, 1024], F16, tag="e")
            for kh in range(2):
                nc.scalar.activation(e[:, kh * 512:(kh + 1) * 512],
                                     sc[:, kh * 512:(kh + 1) * 512], AF.Exp,
                                     bias=shiftb[:], scale=1.0)
            eT = work.tile([128, 1024], F16, tag="eT")
            for kt in range(NKT):
                pt = ptrans.tile([128, 128], F16, tag="pt")
                nc.tensor.transpose(pt[:], e[:, kt * 128:(kt + 1) * 128], idt[:])
                nc.vector.tensor_copy(eT[:, kt * 128:(kt + 1) * 128], pt[:])
            po = pout.tile([128, 512], F32, tag="po")
            psv = pout.tile([128, 1], F32, tag="ps")
            for kt in range(NKT):
                nc.tensor.matmul(
                    psv[:], eT[:, kt * 128:(kt + 1) * 128], onest[:, kt:kt + 1],
                    start=(kt == 0), stop=(kt == NKT - 1),
                )
            for kt in range(NKT):
                nc.tensor.matmul(
                    po[:], eT[:, kt * 128:(kt + 1) * 128],
                    vts[:, kt * D:(kt + 1) * D],
                    start=(kt == 0), stop=(kt == NKT - 1),
                )
            rs = work.tile([128, 1], F32, tag="rs")
            nc.vector.reciprocal(rs[:], psv[:])
            ob = work.tile([128, D], F32, tag="ob")
            nc.vector.tensor_scalar_mul(ob[:], po[:], rs[:])
            nc.sync.dma_start(out[0], ob[:])

        for _rep in range(reps):
            one_rep()


@functools.cache
def _get_fn(reps=1):
    import jax
    from jax.sharding import Mesh, PartitionSpec as P
    import concourse.tile as tile
    import concourse.mybir as mybir
    from concourse.bass2jax import bass_jit, bass_shard_map

    @bass_jit
    def _core(nc, qT, kT, wqT, wkT, wv, v, ones, ident):
        out = nc.dram_tensor("attn_out", [1, QS, D], mybir.dt.float32,
                             kind="ExternalOutput")
        with tile.TileContext(nc) as tc:
            _emit(nc, tc, out, qT[:], kT[:], wqT[:], wkT[:], wv[:], v[:],
                  ones[:], ident[:], dict(), reps=reps)
        return out

    devs = jax.devices()[:NCORES]
    mesh = Mesh(np.array(devs), ("core",))
    fn = bass_shard_map(_core, mesh=mesh, in_specs=(P("core"),) * 8,
                        out_specs=P("core"))
    return fn, mesh


def _prep(queries, keys, values, valid_lens, Wq, Wk, wv):
    """Host-side layout prep. Returns the 8-way stacked input arrays."""
    queries = np.ascontiguousarray(np.asarray(queries, np.float32))
    keys = np.ascontiguousarray(np.asarray(keys, np.float32))
    values = np.ascontiguousarray(np.asarray(values, np.float32))
    vl = np.asarray(valid_lens).astype(np.int64)
    Wq = np.asarray(Wq, np.float32)
    Wk = np.asarray(Wk, np.float32)
    wv = np.asarray(wv, np.float32)

    qT_s = np.stack([
        np.ascontiguousarray(queries[c // 2, (c % 2) * QS:(c % 2 + 1) * QS, :].T
                             ).reshape(NDT, 128, QS)
        for c in range(NCORES)
    ])
    kT_b = [np.ascontiguousarray(keys[b].T).reshape(NDT, 128, K) for b in range(B)]
    kT_s = np.stack([kT_b[c // 2] for c in range(NCORES)])

    WqT = np.ascontiguousarray(Wq.T).reshape(NDT, 128, H)
    WkT = np.ascontiguousarray(Wk.T).reshape(NDT, 128, H)
    wqT_s = np.broadcast_to(WqT, (NCORES, NDT, 128, H))
    wkT_s = np.broadcast_to(WkT, (NCORES, NDT, 128, H))

    wv_sb = np.zeros((128, 2048), np.float16)
    for t in range(2):
        for c in range(32):
            wv_sb[:, (t * 32 + c) * 32 + c] = wv[t * 128:(t + 1) * 128].astype(np.float16)
    wv_s = np.broadcast_to(wv_sb, (NCORES, 128, 2048))

    v_b, ones_b = [], []
    for b in range(B):
        Vm = values[b].astype(np.float16)
        Vm[int(vl[b]):] = 0.0
        v_b.append(Vm.reshape(NKT, 128, D))
        idx = np.arange(K).reshape(NKT, 128).T  # (128, NKT): idx[p, j] = j*128+p
        ones_b.append((idx < int(vl[b])).astype(np.float16))
    v_s = np.stack([v_b[c // 2] for c in range(NCORES)])
    ones_s = np.stack([ones_b[c // 2] for c in range(NCORES)])

    ident_s = np.broadcast_to(np.eye(128, dtype=np.float16), (NCORES, 128, 128))

    return [np.ascontiguousarray(a) for a in
            (qT_s, kT_s, wqT_s, wkT_s, wv_s, v_s, ones_s, ident_s)]


def _gather(out_s):
    # out_s: (NCORES, QS, D); core c = (b = c//2, qh = c%2)
    return np.asarray(out_s).reshape(B, 2 * QS, D).astype(np.float32)


def kernel(**inputs) -> np.ndarray:
    fn, _ = _get_fn()
    args = _prep(**inputs)
    out = fn(*args)
    return _gather(out)


def _time_fn(fn, dargs, iters, warmup):
    import time
    import jax
    out = None
    for _ in range(warmup):
        out = fn(*dargs)
        jax.block_until_ready(out)
    times = []
    for _ in range(iters):
        t0 = time.perf_counter()
        r = fn(*dargs)
        jax.block_until_ready(r)
        times.append(time.perf_counter() - t0)
    return out, times


def run_timed(inputs, iters=30, warmup=2, reps_hi=11):
    """Estimate the per-invocation device time by the two-NEFF delta method:
    the axon dispatch overhead (~76 ms) is identical for both programs, so
    (T(reps_hi) - T(1)) / (reps_hi - 1) isolates one compute repetition.
    Returns (output, body_seconds, details_dict)."""
    import jax
    from jax.sharding import NamedSharding, PartitionSpec as P

    fn1, mesh = _get_fn(reps=1)
    fnR, _ = _get_fn(reps=reps_hi)
    args = _prep(**inputs)
    sh = NamedSharding(mesh, P("core"))
    dargs = [jax.device_put(a, sh) for a in args]

    out, t1 = _time_fn(fn1, dargs, iters, warmup)
    outR, tR = _time_fn(fnR, dargs, iters, warmup)
    assert np.allclose(np.asarray(out), np.asarray(outR)), "reps output mismatch"

    body = (min(tR) - min(t1)) / (reps_hi - 1)
    det = dict(t1_min=min(t1), t1_med=sorted(t1)[len(t1) // 2],
               tR_min=min(tR), tR_med=sorted(tR)[len(tR) // 2], body=body)
    return _gather(out), body, det


# revision 26
# speedup vs baseline: 3.2652x; 3.2652x over previous
"""Additive (Bahdanau) attention on 8 Trainium2 NeuronCores.

Problem:  out[b,q,v] = softmax_k( sum_h wv[h] * tanh(qp[b,q,h] + kp[b,k,h]) ) @ V
          qp = queries @ Wq.T, kp = keys @ Wk.T, key-axis mask from valid_lens.
Shapes:   B=4, Q=256, K=1024, D=512, H=256.

Sharding: 8 shards = (batch b, query-half qh); each core handles 128 queries of
one batch. All projections/tanh/softmax/AV run on-device; the host only slices,
transposes, casts and zero-pads inputs (layout prep).

Device algorithm (per core):
  - qp (h-part, q-free) and kp (h-part, k-free) via fp32 PE matmuls (inputs
    fed d-major so the contraction dim sits on partitions).
  - For each query q and each h-tile t: ScalarE computes
    feats = tanh(kp_t + qp_t[:, q])  in ONE activation instruction
    (the per-partition `bias` operand does the broadcast add for free),
    writing fp16 (ScalarE rate is dtype-independent; fp16 keeps the PE
    score matmuls at full 1 cycle/row instead of fp32's 4).
  - PE reduces over h with a zero-padded fp16 wv weight tile: lhsT is
    (128h, 32) holding wv_t in column q%32, so the scores land in PSUM row q
    (output base partition q//32*32, 32-aligned as the PE requires). Matmuls
    accumulate into a (128q, 1024k) fp32 PSUM scores tile; zero weight
    columns contribute nothing to the other 31 rows of the strip.
  - softmax without max-subtraction: e = exp(scores - 6) in fp16.
    |score| <= sum|wv| ~ 13, so e <= e^7 ~ 1100 fits fp16; the constant
    shift cancels exactly in the normalization. Masking is folded into the
    AV step: V rows >= valid_len are zeroed on the host and the denominator
    uses a masked ones-vector, so no mask instructions run on-device.
  - e is PE-transposed (8x 128x128, fp16) and PE computes out = eT.T @ Vm
    and s = eT.T @ ones_masked (fp32 PSUM accumulation); DVE reciprocal +
    per-partition scale finish in fp32.
"""

import functools
import numpy as np

B, Q, K, D, H = 4, 256, 1024, 512, 256
NCORES = 8
QS = Q // 2  # queries per core (128)
NDT = 4      # d-tiles of 128 in D=512
NKT = K // 128
ESHIFT = 6.0  # exp(score - ESHIFT); cancels in normalization


def _emit(nc, tc, out, qT, kT, wqT, wkT, wv, v, ones, ident, cfg, reps=1):
    import concourse.mybir as mybir

    F32 = mybir.dt.float32
    F16 = mybir.dt.float16
    AF = mybir.ActivationFunctionType

    with (
        tc.tile_pool(name="consts", bufs=1) as consts,
        tc.tile_pool(name="work", bufs=2) as work,
        tc.tile_pool(name="feats", bufs=6) as featsp,
        tc.tile_pool(name="pkp", bufs=1, space="PSUM") as pkp,
        tc.tile_pool(name="pscore", bufs=1, space="PSUM") as pscore,
        tc.tile_pool(name="psmall", bufs=2, space="PSUM") as psmall,
    ):
        # ---- load everything into SBUF (once) ----
        # DMA triggers are ~0.65us each on a sequencer; spread them across
        # sync (keys: critical path), vector (weights/queries: needed for the
        # projections) and gpsimd (tail-only tensors) so they issue in
        # parallel instead of forming a ~13us serial train.
        kts = consts.tile([128, NDT * K], F16)
        for d in range(NDT):
            nc.sync.dma_start(kts[:, d * K:(d + 1) * K], kT[0, d])
        wkts = consts.tile([128, NDT * H], F16)
        for d in range(NDT):
            nc.scalar.dma_start(wkts[:, d * H:(d + 1) * H], wkT[0, d])
        qts = consts.tile([128, NDT * QS], F16)
        for d in range(NDT):
            nc.scalar.dma_start(qts[:, d * QS:(d + 1) * QS], qT[0, d])
        wqts = consts.tile([128, NDT * H], F16)
        for d in range(NDT):
            nc.scalar.dma_start(wqts[:, d * H:(d + 1) * H], wqT[0, d])
        wvt = consts.tile([128, 2048], F16)
        nc.sync.dma_start(wvt[:], wv[0])
        vts = consts.tile([128, NKT * D], F16)
        for kt in range(NKT):
            nc.gpsimd.dma_start(vts[:, kt * D:(kt + 1) * D], v[0, kt])
        onest = consts.tile([128, NKT], F16)
        nc.gpsimd.dma_start(onest[:], ones[0])
        idt = consts.tile([128, 128], F16)
        nc.gpsimd.dma_start(idt[:], ident[0])
        shiftb = consts.tile([128, 1], F32)
        nc.gpsimd.memset(shiftb[:], -ESHIFT)
        # Dummy activation: pulls the exp_and_others ACT table load (~1.3us)
        # off the critical path -- it runs at kernel start instead of right
        # before the first dependent tanh.
        dummy = consts.tile([128, 1], F32)
        nc.scalar.activation(dummy[:], shiftb[:], AF.Tanh)

        def one_rep():
            # ---- projections (fp16 in, fp32 PSUM out) ----
            # kp stays resident in PSUM (ScalarE reads PSUM with lower
            # per-instruction overhead than SBUF); qp is copied to SBUF
            # because the activation bias operand must be SBUF.
            kp = [pkp.tile([128, K], F32, tag=f"kp{t}", name=f"kp{t}")
                  for t in range(2)]
            qp = work.tile([128, 2 * QS], F32, tag="qp")

            def kp_proj_kh(t, kh):
                for d in range(NDT):
                    nc.tensor.matmul(
                        kp[t][:, kh * 512:(kh + 1) * 512],
                        wkts[:, d * H + t * 128: d * H + (t + 1) * 128],
                        kts[:, d * K + kh * 512: d * K + (kh + 1) * 512],
                        start=(d == 0), stop=(d == NDT - 1),
                    )

            def kp_proj(t):
                kp_proj_kh(t, 0)
                kp_proj_kh(t, 1)

            def qp_proj():
                for t in range(2):
                    ps = psmall.tile([128, 512], F32, tag="sm")
                    for d in range(NDT):
                        nc.tensor.matmul(
                            ps[:, :QS],
                            wqts[:, d * H + t * 128: d * H + (t + 1) * 128],
                            qts[:, d * QS:(d + 1) * QS],
                            start=(d == 0), stop=(d == NDT - 1),
                        )
                    nc.vector.tensor_copy(qp[:, t * QS:(t + 1) * QS],
                                          ps[:, :QS])

            qp_proj()
            kp_proj(0)

            # ---- query loop: tanh (fp16 out) + score accumulation (fp32 PSUM)
            sc = pscore.tile([128, 1024], F32, tag="sc")  # 2 PSUM banks

            def do_qt(q, t):
                j, c = q // 32, q % 32
                ft = featsp.tile([128, K], F16, tag="ft")
                nc.scalar.activation(
                    ft[:], kp[t][:], AF.Tanh,
                    bias=qp[:, t * QS + q: t * QS + q + 1], scale=1.0,
                )
                w = wvt[:, (t * 32 + c) * 32: (t * 32 + c) * 32 + 32]
                for kh in range(2):
                    nc.tensor.matmul(
                        sc[32 * j: 32 * j + 32, kh * 512:(kh + 1) * 512],
                        w,
                        ft[:, kh * 512:(kh + 1) * 512],
                        start=(q % 32 == 0 and t == 0),
                        stop=(q % 32 == 31 and t == 1),
                        skip_group_check=True,
                        tile_position=(0, 32 * j),
                    )

            # t=0-only warmup for the first few queries: their tanh stream
            # hides the kp[1] projection (kp_proj(1) is emitted after the
            # warmup score matmuls so the in-order PE consumes the warmup
            # ft tiles first).
            WARM = 8
            for q in range(WARM):
                do_qt(q, 0)
            kp_proj(1)
            for q in range(WARM):
                do_qt(q, 1)
            for q in range(WARM, QS):
                do_qt(q, 0)
                do_qt(q, 1)

            # ---- softmax (shifted, no max-subtraction) + AV ----
            e = work.tile([128, 1024], F16, tag="e")
            for kh in range(2):
                nc.scalar.activation(e[:, kh * 512:(kh + 1) * 512],
                                     sc[:, kh * 512:(kh + 1) * 512], AF.Exp,
                                     bias=shiftb[:], scale=1.0)
            eT = work.tile([128, 1024], F16, tag="eT")
            for kt in range(NKT):
                pt = psmall.tile([128, 512], F16, tag="sm")
                nc.tensor.transpose(pt[:, :128], e[:, kt * 128:(kt + 1) * 128],
                                    idt[:])
                nc.vector.tensor_copy(eT[:, kt * 128:(kt + 1) * 128], pt[:, :128])
            po = psmall.tile([128, 512], F32, tag="sm")
            psv = psmall.tile([128, 512], F32, tag="sm")
            for kt in range(NKT):
                nc.tensor.matmul(
                    psv[:, :1], eT[:, kt * 128:(kt + 1) * 128],
                    onest[:, kt:kt + 1],
                    start=(kt == 0), stop=(kt == NKT - 1),
                )
            for kt in range(NKT):
                nc.tensor.matmul(
                    po[:], eT[:, kt * 128:(kt + 1) * 128],
                    vts[:, kt * D:(kt + 1) * D],
                    start=(kt == 0), stop=(kt == NKT - 1),
                )
            rs = work.tile([128, 1], F32, tag="rs")
            nc.vector.reciprocal(rs[:], psv[:, :1])
            ob = work.tile([128, D], F32, tag="ob")
            nc.vector.tensor_scalar_mul(ob[:], po[:], rs[:])
            nc.sync.dma_start(out[0], ob[:])

        for _rep in range(reps):
            one_rep()


@functools.cache
def _get_fn(reps=1):
    import jax
    from jax.sharding import Mesh, PartitionSpec as P
    import concourse.tile as tile
    import concourse.mybir as mybir
    from concourse.bass2jax import bass_jit, bass_shard_map

    @bass_jit
    def _core(nc, qT, kT, wqT, wkT, wv, v, ones, ident):
        out = nc.dram_tensor("attn_out", [1, QS, D], mybir.dt.float32,
                             kind="ExternalOutput")
        with tile.TileContext(nc) as tc:
            _emit(nc, tc, out, qT[:], kT[:], wqT[:], wkT[:], wv[:], v[:],
                  ones[:], ident[:], dict(), reps=reps)
        return out

    devs = jax.devices()[:NCORES]
    mesh = Mesh(np.array(devs), ("core",))
    fn = bass_shard_map(_core, mesh=mesh, in_specs=(P("core"),) * 8,
                        out_specs=P("core"))
    return fn, mesh


def _prep(queries, keys, values, valid_lens, Wq, Wk, wv):
    """Host-side layout prep. Returns the 8-way stacked input arrays."""
    queries = np.ascontiguousarray(np.asarray(queries, np.float32))
    keys = np.ascontiguousarray(np.asarray(keys, np.float32))
    values = np.ascontiguousarray(np.asarray(values, np.float32))
    vl = np.asarray(valid_lens).astype(np.int64)
    Wq = np.asarray(Wq, np.float32)
    Wk = np.asarray(Wk, np.float32)
    wv = np.asarray(wv, np.float32)

    qT_s = np.stack([
        queries[c // 2, (c % 2) * QS:(c % 2 + 1) * QS, :].T.astype(np.float16)
        .reshape(NDT, 128, QS)
        for c in range(NCORES)
    ])
    kT_b = [keys[b].T.astype(np.float16).reshape(NDT, 128, K) for b in range(B)]
    kT_s = np.stack([kT_b[c // 2] for c in range(NCORES)])

    WqT = Wq.T.astype(np.float16).reshape(NDT, 128, H)
    WkT = Wk.T.astype(np.float16).reshape(NDT, 128, H)
    wqT_s = np.broadcast_to(WqT, (NCORES, NDT, 128, H))
    wkT_s = np.broadcast_to(WkT, (NCORES, NDT, 128, H))

    wv_sb = np.zeros((128, 2048), np.float16)
    for t in range(2):
        for c in range(32):
            wv_sb[:, (t * 32 + c) * 32 + c] = wv[t * 128:(t + 1) * 128].astype(np.float16)
    wv_s = np.broadcast_to(wv_sb, (NCORES, 128, 2048))

    v_b, ones_b = [], []
    for b in range(B):
        Vm = values[b].astype(np.float16)
        Vm[int(vl[b]):] = 0.0
        v_b.append(Vm.reshape(NKT, 128, D))
        idx = np.arange(K).reshape(NKT, 128).T  # (128, NKT): idx[p, j] = j*128+p
        ones_b.append((idx < int(vl[b])).astype(np.float16))
    v_s = np.stack([v_b[c // 2] for c in range(NCORES)])
    ones_s = np.stack([ones_b[c // 2] for c in range(NCORES)])

    ident_s = np.broadcast_to(np.eye(128, dtype=np.float16), (NCORES, 128, 128))

    return [np.ascontiguousarray(a) for a in
            (qT_s, kT_s, wqT_s, wkT_s, wv_s, v_s, ones_s, ident_s)]


def _gather(out_s):
    # out_s: (NCORES, QS, D); core c = (b = c//2, qh = c%2)
    return np.asarray(out_s).reshape(B, 2 * QS, D).astype(np.float32)


def kernel(**inputs) -> np.ndarray:
    fn, _ = _get_fn()
    args = _prep(**inputs)
    out = fn(*args)
    return _gather(out)


def _time_fn(fn, dargs, iters, warmup):
    import time
    import jax
    out = None
    for _ in range(warmup):
        out = fn(*dargs)
        jax.block_until_ready(out)
    times = []
    for _ in range(iters):
        t0 = time.perf_counter()
        r = fn(*dargs)
        jax.block_until_ready(r)
        times.append(time.perf_counter() - t0)
    return out, times


def run_timed(inputs, iters=30, warmup=2, reps_hi=11):
    """Estimate the per-invocation device time by the two-NEFF delta method:
    the axon dispatch overhead (~76 ms) is identical for both programs, so
    (T(reps_hi) - T(1)) / (reps_hi - 1) isolates one compute repetition.
    Returns (output, body_seconds, details_dict)."""
    import jax
    from jax.sharding import NamedSharding, PartitionSpec as P

    fn1, mesh = _get_fn(reps=1)
    fnR, _ = _get_fn(reps=reps_hi)
    args = _prep(**inputs)
    sh = NamedSharding(mesh, P("core"))
    dargs = [jax.device_put(a, sh) for a in args]

    out, t1 = _time_fn(fn1, dargs, iters, warmup)
    outR, tR = _time_fn(fnR, dargs, iters, warmup)
    assert np.allclose(np.asarray(out), np.asarray(outR)), "reps output mismatch"

    body = (min(tR) - min(t1)) / (reps_hi - 1)
    det = dict(t1_min=min(t1), t1_med=sorted(t1)[len(t1) // 2],
               tR_min=min(tR), tR_med=sorted(tR)[len(tR) // 2], body=body)
    return _gather(out), body, det
